# revision 1
# baseline (speedup 1.0000x reference)
"""Trainium2 Bass kernel for nn_MultiHeadAttention (B=2, S=4096, D=512, H=8).

Sharding: 8 cores; core c handles batch b = c//4 and q-row slice (c%4) of
1024 rows, for all 8 heads.  Each core computes its full output rows, so the
host-side gather is a pure concatenation (no reduction).

Per-core dataflow (fp16 matmul datapath, fp32 accumulation):
  - x/y/z slices are loaded fp32 in 512-row chunks, cast to fp16 (DVE),
    bounced through DRAM (gpsimd SWDGE writes, so both HWDGE queues stay
    free) and re-loaded through the DMA xbar transpose to get feature-major
    layouts.  Production is software-pipelined per 1024-row block with the
    load stream emitted one block ahead.
  - qT/kT projections produce [feat, seq] tiles; V is produced in natural
    [seq, feat] layout with an interleaved all-ones column per head (the
    ones column makes the AV matmul emit the softmax denominator Z).
  - scoresT[j, i] = kT^T qT per 128-row j-chunk (two heads packed in the
    PE array via row tiling at base partitions 0/64), exp on the scalar
    engine with the 1/sqrt(HD) scale fused (no max subtraction: scores are
    ~N(0,1), max < ~6, exp stays in fp16 range).
  - AV accumulates over j-chunks in PSUM; row 64 is Z.  Normalization is
    deferred: avT /= Z via reciprocal + DMA partition-broadcast + one DVE
    multiply, then the output projection accumulates all 8 heads (K=64
    chunks) plus a K=1 ones-row matmul that adds the output bias.
  - Engines execute their instruction streams in order, so overlap is set
    by emission order: attention for the first two head pairs of the first
    i-chunk is emitted interleaved with kv-block production (PSUM budget:
    4 score banks + 4 AV banks, with projection psums sharing the score
    slots), and each i-chunk's output projection is emitted one head-pair
    sweep late to keep ScalarE busy across the boundary.
"""

import sys

sys.path.insert(0, "/opt/trn_rl_repo")

import numpy as np

import concourse.bass as bass
import concourse.mybir as mybir
import concourse.tile as tile
from concourse import bacc
from concourse.dve_ops import (
    RECIP_APPROX_FAST_CONSTS as RECIP_CONSTS,
    RECIPROCAL_APPROX_FAST,
)

F16 = mybir.dt.float16
F32 = mybir.dt.float32

B, S, D, H = 2, 4096, 512, 8
HD = D // H  # 64
N_CORES = 8
CORES_PER_B = N_CORES // B  # 4
SI = S // CORES_PER_B  # 1024 q rows per core
VW = HD + 1  # v + ones column
USE_FAST_RECIP = False


def build_mha_nc(s=S, si=SI, d=D, h=H, stop=None):
    """Build the per-core Bass program.  s = kv length, si = q rows.
    stop: one of None/"w"/"bounce"/"proj" to truncate for profiling."""
    hd = d // h
    vw = hd + 1
    hp_n = h // 2  # head pairs
    dc_n = d // 128  # D chunks of 128
    jc_n = s // 128  # kv chunks of 128 rows
    ic_n = max(1, si // 512)  # i chunks of 512
    ic_w = min(si, 512)
    isub_n = ic_w // 128

    nc = bacc.Bacc("TRN2", target_bir_lowering=False, debug=False,
                   num_devices=N_CORES)

    xs = nc.dram_tensor("xs", [si, d], F32, kind="ExternalInput")
    yb = nc.dram_tensor("yb", [s, d], F32, kind="ExternalInput")
    zb = nc.dram_tensor("zb", [s, d], F32, kind="ExternalInput")
    wq = nc.dram_tensor("wq", [d, d], F32, kind="ExternalInput")
    wk = nc.dram_tensor("wk", [d, d], F32, kind="ExternalInput")
    wv = nc.dram_tensor("wv", [d, d], F32, kind="ExternalInput")
    wp = nc.dram_tensor("wp", [d, d], F32, kind="ExternalInput")
    bq = nc.dram_tensor("bq", [1, d], F32, kind="ExternalInput")
    bk = nc.dram_tensor("bk", [1, d], F32, kind="ExternalInput")
    bv = nc.dram_tensor("bv", [1, d], F32, kind="ExternalInput")
    bp = nc.dram_tensor("bp", [1, d], F32, kind="ExternalInput")
    out = nc.dram_tensor("out", [si, d], F32, kind="ExternalOutput")

    mult = mybir.AluOpType.mult
    add = mybir.AluOpType.add
    EXP = mybir.ActivationFunctionType.Exp

    with tile.TileContext(nc) as tc:
        with (
            tc.tile_pool(name="consts", bufs=1) as consts,
            tc.tile_pool(name="persist", bufs=1) as persist,
            tc.tile_pool(name="dram16", bufs=1, space="DRAM") as dram16,
            tc.tile_pool(name="attp", bufs=5) as attp,
            tc.tile_pool(name="avtp", bufs=2) as avtp,
            tc.tile_pool(name="nrm", bufs=2) as nrm,
            tc.tile_pool(name="outp", bufs=2) as outp,
            tc.tile_pool(name="sc_ps", bufs=2, space="PSUM") as sc_ps,
            tc.tile_pool(name="av_ps", bufs=2, space="PSUM") as av_ps,
        ):
            # ---------------- weights / biases -> SBUF (fp16) -------------
            with tc.tile_pool(name="wload", bufs=1) as wload:
                def load_cast_w(wdram, name):
                    # [d, d] -> [128, dc_n, d] f16, chunk c = rows c*128..
                    stg = wload.tile([128, dc_n, d], F32, tag="wstage",
                                     name=f"{name}s")
                    nc.sync.dma_start(stg[:], wdram.ap().rearrange(
                        "(c p) f -> p c f", p=128))
                    wsb = consts.tile([128, dc_n, d], F16, name=name)
                    nc.vector.tensor_copy(wsb[:], stg[:])
                    return wsb

                wq_sb = load_cast_w(wq, "wq_sb")
                wk_sb = load_cast_w(wk, "wk_sb")
                wv_sb = load_cast_w(wv, "wv_sb")

                # wp per-head-aligned: [64, h, d] f16 (head hh = rows hh*hd)
                wps = wload.tile([64, h, d], F32, tag="wstage2", name="wps")
                nc.sync.dma_start(wps[:], wp.ap().rearrange(
                    "(hh p) f -> p hh f", p=hd))
                wp_sb = consts.tile([64, h, d], F16, name="wp_sb")
                nc.vector.tensor_copy(wp_sb[:], wps[:])

                # bq/bk as per-partition scalars [128, dc_n]
                def load_bias_p(bdram, name):
                    t = consts.tile([128, dc_n], F32, name=name)
                    nc.sync.dma_start(t[:], bdram.ap().rearrange(
                        "o (c p) -> (o p) c", p=128))
                    return t

                bq_sb = load_bias_p(bq, "bq_sb")
                bk_sb = load_bias_p(bk, "bk_sb")

                # bv broadcast across partitions [128, d] f32
                bv_sb = consts.tile([128, d], F32, name="bv_sb")
                nc.sync.dma_start(
                    bv_sb[:],
                    bass.AP(bv.ap().tensor, 0, [[1, 1], [0, 128], [1, d]]))

                # bp row (f16) + ones row for the K=1 bias matmul
                bps = wload.tile([1, d], F32, tag="bps", name="bps")
                nc.sync.dma_start(bps[:], bp.ap())
                bp_sb = consts.tile([1, d], F16, name="bp_sb")
                nc.vector.tensor_copy(bp_sb[:], bps[:])
                ones_sb = consts.tile([1, 128], F16, name="ones_sb")
                nc.vector.memset(ones_sb[:], 1.0)

            go_proj = stop not in ("w", "bounce")
            go_attn = go_proj and stop != "proj"
            BLK = min(1024, s, si)

            # persistent projection outputs
            kT = [persist.tile([128, s], F16, name=f"kT{fp}") for fp in range(hp_n)]
            qT = [persist.tile([128, si], F16, name=f"qT{fp}") for fp in range(hp_n)]
            v_ext = [persist.tile([128, h * vw], F16, name=f"vx{sc}")
                     for sc in range(s // 128)]

            y16b = [dram16.tile([BLK, d], F16, name=f"y16_{b}")
                    for b in range(s // BLK)]
            x16b = [dram16.tile([BLK, d], F16, name=f"x16_{b}")
                    for b in range(si // BLK)]
            z16b = [dram16.tile([BLK, d], F16, name=f"z16_{b}")
                    for b in range(s // BLK)]

            # ---------------- attention helpers --------------------------
            # (emitted interleaved with kv production for ic0/hp0; engines
            # execute their streams in order, so emission order IS overlap.)
            def attn_hp(ic, hp, jcs, avA, avB):
                isl = slice(ic * ic_w, (ic + 1) * ic_w)
                for jc in jcs:
                    jsl = slice(jc * 128, (jc + 1) * 128)
                    sc_t = sc_ps.tile([128, 2 * ic_w], F32, tag="sc",
                                      name="sct")
                    nc.tensor.matmul(
                        sc_t[:, 0:ic_w], kT[hp][0:64, jsl],
                        qT[hp][0:64, isl], start=True, stop=True)
                    nc.tensor.matmul(
                        sc_t[:, ic_w:2 * ic_w], kT[hp][64:128, jsl],
                        qT[hp][64:128, isl], start=True, stop=True)
                    att = attp.tile([128, 2 * ic_w], F16, tag="att",
                                    name="att")
                    nc.scalar.activation(att[:], sc_t[:], EXP,
                                         scale=1.0 / np.sqrt(hd))
                    hA, hB = 2 * hp, 2 * hp + 1
                    nc.tensor.matmul(
                        avA[0:vw, :], v_ext[jc][:, hA * vw:(hA + 1) * vw],
                        att[:, 0:ic_w],
                        start=(jc == 0), stop=(jc == jc_n - 1))
                    nc.tensor.matmul(
                        avB[0:vw, :], v_ext[jc][:, hB * vw:(hB + 1) * vw],
                        att[:, ic_w:2 * ic_w],
                        start=(jc == 0), stop=(jc == jc_n - 1))

            def attn_norm(ic, hp, avA, avB, avts):
                for hl, av in ((0, avA), (1, avB)):
                    zr = nrm.tile([1, ic_w], F32, tag="zr", name="zr")
                    if USE_FAST_RECIP:
                        nc.vector._custom_dve(
                            RECIPROCAL_APPROX_FAST, out=zr[:],
                            in0=av[hd:hd + 1, :],
                            s0=RECIP_CONSTS["s0"], s1=RECIP_CONSTS["s1"],
                            imm2=RECIP_CONSTS["imm2"])
                    else:
                        nc.vector.reciprocal(zr[:], av[hd:hd + 1, :])
                    zbc = nrm.tile([64, ic_w], F32, tag="zbc", name="zbc")
                    nc.sync.dma_start(
                        zbc[:],
                        bass.AP(zr.tensor, zr.offset,
                                [[1, 1], [0, 64], [1, ic_w]]))
                    avt = avtp.tile([64, ic_w], F16, tag=f"avt{hp}{hl}",
                                    name=f"avt{hp}{hl}")
                    nc.vector.tensor_tensor(avt[:], av[0:hd, :], zbc[:],
                                            op=mult)
                    avts[2 * hp + hl] = avt

            def out_proj(ic, avts):
                for isub in range(isub_n):
                    ssl = slice(isub * 128, (isub + 1) * 128)
                    po = av_ps.tile([128, d], F32,
                                    tag=("avA", "avB")[isub % 2], name="pot")
                    for hh in range(h):
                        nc.tensor.matmul(po[:], avts[hh][:, ssl],
                                         wp_sb[:, hh, :],
                                         start=(hh == 0), stop=False)
                    nc.tensor.matmul(po[:], ones_sb[:, 0:128], bp_sb[:],
                                     start=False, stop=True)
                    ob = outp.tile([128, d], F32, tag="ob", name="ob")
                    nc.vector.tensor_copy(ob[:], po[:])
                    nc.sync.dma_start(
                        out.ap()[ic * ic_w + isub * 128:
                                 ic * ic_w + (isub + 1) * 128, :], ob[:])

            with (
                tc.tile_pool(name="bnc", bufs=6) as bnc,
                tc.tile_pool(name="tpose", bufs=2) as tpose,
            ):
                CH = min(512, BLK)

                def load_cast_block(src_ap, row0):
                    # fp32 HBM -> SBUF (512-row chunks) -> f16 (DVE cast)
                    s16s = []
                    for ch in range(BLK // CH):
                        stg = bnc.tile([128, CH // 128, d], F32, tag="bstage",
                                       name="bstg", bufs=4)
                        r0 = row0 + ch * CH
                        nc.sync.dma_start(stg[:], src_ap[r0:r0 + CH, :]
                                          .rearrange("(c p) f -> p c f", p=128))
                        s16 = bnc.tile([128, CH // 128, d], F16, tag="bstage16",
                                       name="bstg16", bufs=6)
                        nc.vector.tensor_copy(s16[:], stg[:])
                        s16s.append(s16)
                    return s16s

                def write_block(s16s, dst16):
                    # f16 SBUF -> DRAM via gpsimd SWDGE (own DMA path; keeps
                    # the HWDGE queues free).
                    for ch, s16 in enumerate(s16s):
                        nc.gpsimd.dma_start(
                            dst16[ch * CH:(ch + 1) * CH, :]
                            .rearrange("(c p) f -> p c f", p=128), s16[:])

                def tpose_block(dst16):
                    # xbar-transposed reads, emitted two items behind the
                    # loads so their in-queue wait on the SWDGE writes is
                    # already satisfied and the SP load stream never stalls.
                    aT = []
                    for c in range(dc_n):
                        t = tpose.tile([128, BLK], F16, tag=f"aT{c}",
                                       name=f"aT{c}")
                        nc.sync.dma_start(t[:],
                                          dst16[:, c * 128:(c + 1) * 128],
                                          transpose=True)
                        aT.append(t)
                    return aT

                def proj_block(dst_list, aT, bias_sb, w_sb, row0):
                    # dst[fp][f, block-range] = W[:, fp].T @ actT (+ bias)
                    for sc8 in range(BLK // 512):
                        gsl = slice(row0 + sc8 * 512, row0 + (sc8 + 1) * 512)
                        lsl = slice(sc8 * 512, (sc8 + 1) * 512)
                        for fp in range(hp_n):
                            ps = sc_ps.tile([128, 512], F32, tag="sc",
                                            name="prjps")
                            for c in range(dc_n):
                                nc.tensor.matmul(
                                    ps[:],
                                    w_sb[:, c, fp * 128:(fp + 1) * 128],
                                    aT[c][:, lsl],
                                    start=(c == 0), stop=(c == dc_n - 1))
                            nc.vector.tensor_scalar_add(
                                dst_list[fp][:, gsl], ps[:],
                                bias_sb[:, fp:fp + 1])

                def v_block(aT, row0):
                    for scl in range(BLK // 128):
                        sc = row0 // 128 + scl
                        ps = sc_ps.tile([128, 512], F32, tag="sc",
                                        name="vps")
                        for c in range(dc_n):
                            nc.tensor.matmul(
                                ps[:], aT[c][:, scl * 128:(scl + 1) * 128],
                                wv_sb[:, c, :],
                                start=(c == 0), stop=(c == dc_n - 1))
                        vx = v_ext[sc]
                        nc.vector.memset(vx[:], 1.0)
                        nc.vector.tensor_tensor(
                            vx.rearrange("p (hh e) -> p hh e", e=vw)[:, :, 0:hd],
                            ps.rearrange("p (hh e) -> p hh e", e=hd),
                            bv_sb.rearrange("p (hh e) -> p hh e", e=hd),
                            op=add)

                av00 = av01 = av10 = av11 = None
                prod_hps = min(2, hp_n)
                if go_attn:
                    av00 = av_ps.tile([128, ic_w], F32, tag="avA", name="avA")
                    av01 = av_ps.tile([128, ic_w], F32, tag="avB", name="avB")
                    if prod_hps > 1:
                        av10 = av_ps.tile([128, ic_w], F32, tag="avA",
                                          name="avA")
                        av11 = av_ps.tile([128, ic_w], F32, tag="avB",
                                          name="avB")

                if stop != "w":
                    # x first (qT is needed by every score matmul), then z/y
                    # interleaved; attention(ic0, hp0/hp1) chunks ride right
                    # behind each kv block so ScalarE starts exp'ing early.
                    def y_fin(aT, r):
                        proj_block(kT, aT, bk_sb, wk_sb, r)
                        if go_attn:
                            jcs = range(r // 128, r // 128 + BLK // 128)
                            attn_hp(0, 0, jcs, av00, av01)
                            if prod_hps > 1:
                                attn_hp(0, 1, jcs, av10, av11)

                    work = []
                    for b in range(si // BLK):
                        work.append((xs.ap(), x16b[b], b * BLK,
                                     lambda aT, r: proj_block(qT, aT, bq_sb,
                                                              wq_sb, r)))
                    for b in range(s // BLK):
                        work.append((zb.ap(), z16b[b], b * BLK,
                                     lambda aT, r: v_block(aT, r)))
                        work.append((yb.ap(), y16b[b], b * BLK, y_fin))
                    # software-pipelined emission: loads lead writes by
                    # one item and transposes+consumers by two.
                    wq_, tq_ = [], []

                    def _flush_write():
                        s16s, dst16, row0, fin = wq_.pop(0)
                        write_block(s16s, dst16)
                        tq_.append((dst16, row0, fin))

                    def _flush_tpose():
                        dst16, row0, fin = tq_.pop(0)
                        if go_proj:
                            fin(tpose_block(dst16), row0)

                    for item in work:
                        src_ap, dst16, row0, fin = item
                        s16s = load_cast_block(src_ap, row0)
                        wq_.append((s16s, dst16, row0, fin))
                        if len(wq_) >= 2:
                            _flush_write()
                        if len(tq_) >= 1:
                            _flush_tpose()
                    while wq_:
                        _flush_write()
                    while tq_:
                        _flush_tpose()

                # ---------------- rest of attention ----------------------
                if go_attn:
                    # flat sweep list: remaining (ic, hp) pairs in order;
                    # each ic's out_proj is emitted one sweep late so the
                    # scalar engine keeps exp'ing across the boundary.
                    avts_by_ic = [[None] * h for _ in range(ic_n)]
                    attn_norm(0, 0, av00, av01, avts_by_ic[0])
                    if prod_hps > 1:
                        attn_norm(0, 1, av10, av11, avts_by_ic[0])
                    sweeps = [(0, hp) for hp in range(prod_hps, hp_n)]
                    for ic in range(1, ic_n):
                        sweeps += [(ic, hp) for hp in range(hp_n)]
                    pending_proj = None
                    for ic, hp in sweeps:
                        avA = av_ps.tile([128, ic_w], F32, tag="avA",
                                         name="avA")
                        avB = av_ps.tile([128, ic_w], F32, tag="avB",
                                         name="avB")
                        attn_hp(ic, hp, range(jc_n), avA, avB)
                        if pending_proj is not None:
                            out_proj(*pending_proj)
                            pending_proj = None
                        attn_norm(ic, hp, avA, avB, avts_by_ic[ic])
                        if hp == hp_n - 1:
                            pending_proj = (ic, avts_by_ic[ic])
                    if pending_proj is not None:
                        out_proj(*pending_proj)

    nc.finalize()
    return nc


_NC_CACHE = {}


def _get_nc():
    if "nc" not in _NC_CACHE:
        _NC_CACHE["nc"] = build_mha_nc()
    return _NC_CACHE["nc"]


def kernel(x, y, z, Wq, bq, Wk, bk, Wv, bv, Wp, bp):
    x = np.ascontiguousarray(np.asarray(x, np.float32))
    y = np.ascontiguousarray(np.asarray(y, np.float32))
    z = np.ascontiguousarray(np.asarray(z, np.float32))
    ws = {n: np.ascontiguousarray(np.asarray(a, np.float32))
          for n, a in (("wq", Wq), ("wk", Wk), ("wv", Wv), ("wp", Wp))}
    bs = {n: np.ascontiguousarray(np.asarray(a, np.float32).reshape(1, D))
          for n, a in (("bq", bq), ("bk", bk), ("bv", bv), ("bp", bp))}

    from concourse.bass_utils import run_bass_kernel_spmd

    nc = _get_nc()
    in_maps = []
    for c in range(N_CORES):
        b = c // CORES_PER_B
        sl = c % CORES_PER_B
        in_maps.append({
            "xs": np.ascontiguousarray(x[b, sl * SI:(sl + 1) * SI, :]),
            "yb": y[b], "zb": z[b], **ws, **bs,
        })
    res = run_bass_kernel_spmd(nc, in_maps, core_ids=list(range(N_CORES)))
    outa = np.empty((B, S, D), np.float32)
    for c in range(N_CORES):
        b = c // CORES_PER_B
        sl = c % CORES_PER_B
        outa[b, sl * SI:(sl + 1) * SI, :] = res.results[c]["out"]
    return outa

